# revision 19
# baseline (speedup 1.0000x reference)
"""Trainium2 (Bass) SPMD kernel for the CBGNN message-passing problem. v6.

Structure (per core, 8-way SPMD):
  A: per-cycle MLP scores for its 32768 cycles -> 0.5MB AllGather (out_tab).
  B: seg0-sorted edge stream [128 x 4224] (fp16): indirect-gather
     out_tab[seg1], exp-weighted masked scans -> segment softmax
     numerator/denominator, extracted at segment-end slots -> o2 ->
     0.5MB fp16 AllGather (out2_tab).
  C: target-aligned row-balanced Edge2cycle stream: indirect-gather
     out2_tab[src], add-reset max-scan -> per-target maxima in LOCAL DRAM.
  D: LayerNorm makes MLP2 scale-invariant in its 2-dim input, so its
     output is a function of the input angle only: host precomputes a
     16K-entry table g(t) from the weights; the device computes a
     monotone angle coordinate t(om, wk) per target, gathers g, applies
     the global L2 norm (16B AllReduce) and sigmoid.  All targets of the
     core's contiguous range take this path (om==0 empties included).

v6 vs v5: phase D's 36864-token full MLP + empty fast path replaced by
the angle-table path over the dense per-core target range (~20x less
phase-D work); B/C streams and out2_tab in fp16 (no +64 shift; max-scan
reset via -60000 addend).
"""

import sys

import numpy as np

for _p in ("/opt/trn_rl_repo",):
    if _p not in sys.path:
        sys.path.insert(0, _p)

NCORES = 8
P = 128
ELEM = 64


class Cfg:
    n_cyc = 262144
    out_dim = 256
    e_cc = 4194304
    m_e2c = 4194304
    len_edges = 1048576
    t1 = 4224            # phase-B stream columns per partition row
    t2 = 4224            # phase-C stream columns per partition row
    mcols = 1056         # gather chunk width (stream columns)
    a_mac = 2048         # phase-A macro tile (tokens)
    dcols = 292          # phase-D dense targets per partition row
    n_tab = 16384        # phase-D angle table entries
    neg_slope = 0.2
    ln_eps = 1e-5
    mlp_bf16 = True
    a_fp8 = True         # phase-A z-matmul in fp8 DoubleRow
    act_lrelu = True     # False: DVE max(x, ax) (CoreSim lacks Lrelu)
    phases = "abcd"
    repeat = 1           # timing aid: repeat phase pipeline inside one NEFF

    @property
    def seg_pc(self):
        return self.n_cyc // NCORES

    @property
    def toka(self):
        return self.n_cyc // NCORES


class SmallCfg(Cfg):
    n_cyc = 16384
    e_cc = 65536
    m_e2c = 65536
    len_edges = 16384
    t1 = 96
    t2 = 96
    mcols = 48
    a_mac = 2048
    dcols = 20


# ---------------------------------------------------------------------------
# host-side sharding / layout (index work + parameter folding only)
# ---------------------------------------------------------------------------

def _pack_rows(counts, ncols):
    csum = np.cumsum(counts)
    nseg = len(counts)
    seg_row = np.empty(nseg, np.int64)
    seg_col0 = np.empty(nseg, np.int64)
    start = 0
    base = 0
    for r in range(P):
        j = int(np.searchsorted(csum, base + ncols, side="right"))
        if j < nseg and counts[j] > ncols:
            raise ValueError("segment larger than a row")
        prev = base
        seg_row[start:j] = r
        seg_col0[start:j] = (csum[start:j] - counts[start:j]) - prev
        if j > 0:
            base = int(csum[j - 1])
        start = j
        if start == nseg:
            break
    if start != nseg:
        raise ValueError("edges did not fit into P rows")
    return seg_row, seg_col0


def _layout_stream(named_vals, seg_local, seg_counts, seg_starts_local, ncols):
    seg_row, seg_col0 = _pack_rows(seg_counts, ncols)
    rank = np.arange(len(seg_local)) - seg_starts_local[seg_local]
    slot = seg_row[seg_local] * ncols + seg_col0[seg_local] + rank
    out = {}
    for name, (vals, fill) in named_vals.items():
        arr = np.full(P * ncols, fill, dtype=np.asarray(vals).dtype)
        arr[slot] = vals
        out[name] = arr.reshape(P, ncols)
    lab = np.full(P * ncols, -1, np.int64)
    lab[slot] = seg_local
    lab2 = lab.reshape(P, ncols)
    msk = np.zeros((P, ncols), np.float32)
    msk[:, 1:] = ((lab2[:, 1:] == lab2[:, :-1]) & (lab2[:, 1:] >= 0)).astype(
        np.float32)
    out["__mask"] = msk
    end_slot = seg_row * ncols + seg_col0 + seg_counts - 1
    filler_slot = P * ncols - 1
    assert lab.reshape(-1)[filler_slot] == -1, "last slot is not filler"
    out["__end"] = np.where(seg_counts > 0, end_slot,
                            filler_slot).astype(np.int64)
    return out


def _fold_mlp(W1, g, b, W2, b2):
    W2 = np.asarray(W2, np.float64).reshape(-1)
    g = np.asarray(g, np.float64)
    b = np.asarray(b, np.float64)
    w2eff = g * W2
    b2eff = float(np.asarray(b2).reshape(-1)[0]) + float(np.dot(b, W2))
    return (w2eff.astype(np.float32), np.float32(b2eff),
            np.float32(w2eff.sum()))


def _wmlp64(h, W1, g, b, W2, b2, neg_slope, eps):
    h = h @ np.asarray(W1, np.float64)
    h = np.where(h > 0, h, neg_slope * h)
    mu = h.mean(-1, keepdims=True)
    var = ((h - mu) ** 2).mean(-1, keepdims=True)
    h = (h - mu) / np.sqrt(var + eps) * np.asarray(g, np.float64) \
        + np.asarray(b, np.float64)
    return h @ np.asarray(W2, np.float64) + np.asarray(b2, np.float64)


def host_prepare(inputs, cfg):
    n_cyc, seg_pc = cfg.n_cyc, cfg.seg_pc
    od = cfg.out_dim

    x = np.asarray(inputs["x"], np.float32)
    e2c = np.asarray(inputs["Edge2cycle"])
    eidx = np.asarray(inputs["edge_index"])
    pce = np.asarray(inputs["permuteCE"], np.float32)
    wk = np.asarray(inputs["whether_k"], np.float32)
    assert float(wk.min()) >= 0.0, "angle table assumes whether_k >= 0"

    seg0 = np.asarray(eidx[0], np.int64)
    seg1 = np.asarray(eidx[1], np.int64)

    xT = np.ascontiguousarray(x.T)

    def tab_pos(g):
        return g

    order0 = np.argsort(seg0, kind="stable")
    seg0s = seg0[order0]
    cnt0 = np.bincount(seg0, minlength=n_cyc).astype(np.int64)
    start0 = np.zeros(n_cyc + 1, np.int64)
    np.cumsum(cnt0, out=start0[1:])

    tgt = np.asarray(e2c[:, 0], np.int64)
    src = np.asarray(e2c[:, 1], np.int64)
    order1 = np.argsort(tgt, kind="stable")
    tgts = tgt[order1]
    cnt1 = np.bincount(tgt, minlength=cfg.len_edges).astype(np.int64)
    start1 = np.zeros(cfg.len_edges + 1, np.int64)
    np.cumsum(cnt1, out=start1[1:])

    # ---- parameter folding (phase A MLP)
    w2eff, b2e, s2 = _fold_mlp(inputs["W1"], inputs["g1"], inputs["b1"],
                               inputs["W2"], inputs["b2"])
    w1_h = np.asarray(inputs["W1"], np.float32)

    cst = np.zeros(16, np.float32)
    cst[0], cst[1] = -s2, b2e
    cst[4] = np.float32(cfg.ln_eps)
    # wpack layout: [0:4) ow2 (col 2*jh+0 = 1.0, col 2*jh+1 =
    # w2eff[jh*128+p]) | [4:20) cst.  W1 ships separately in fp8
    # (DoubleRow layout, pre-scaled by SW; the ACT Lrelu un-scales).
    SW = 32.0
    wcols = 4 + 16
    wpack = np.zeros((P, wcols), np.float32)
    for jh in range(2):
        wpack[:, 2 * jh] = 1.0
        wpack[:, 2 * jh + 1] = w2eff[jh * P:(jh + 1) * P]
    wpack[:, 4:20] = np.broadcast_to(cst, (P, 16))
    import ml_dtypes
    if cfg.a_fp8:
        wblk8 = np.zeros((P, 2, 2 * P), np.float32)
        for kin in range(2):
            for jh in range(2):
                wblk8[:, kin, jh * P:(jh + 1) * P] = \
                    w1_h[kin * P:(kin + 1) * P, jh * P:(jh + 1) * P] * SW
        wblk8 = wblk8.reshape(P, 4 * P).astype(ml_dtypes.float8_e4m3)
    else:
        wblk8 = np.zeros((P, 4 * P), np.float32)
        for kin in range(2):
            for jh in range(2):
                b = (kin * 2 + jh) * P
                wblk8[:, b:b + P] = w1_h[kin * P:(kin + 1) * P,
                                         jh * P:(jh + 1) * P]
        wblk8 = wblk8.astype(np.float16)

    # ---- phase D angle table: MLP2 output as a function of input angle
    nt = cfg.n_tab
    tk = np.arange(nt, dtype=np.float64) / (nt - 1)
    u = np.where(tk <= 0.5, 2 * tk, 2 * (1 - tk))
    om_t = np.where(tk <= 0.5, 1 - u, -(1 - u))
    feat_t = np.stack([om_t, u], 1)
    g_tab = _wmlp64(feat_t, inputs["Wk1"], inputs["gk"], inputs["bk"],
                    inputs["Wk2"], inputs["bk2"], cfg.neg_slope,
                    cfg.ln_eps).reshape(-1).astype(np.float32)
    # all-empty tail targets (>= n_cyc): om == 0 exactly, so by scale
    # invariance every one shares the value g0 = g(om=0, wk=1); their L2
    # contribution is closed-form and one synthetic token computes the
    # sigmoid on device.
    g0 = float(_wmlp64(np.array([[0.0, 1.0]]), inputs["Wk1"], inputs["gk"],
                       inputs["bk"], inputs["Wk2"], inputs["bk2"],
                       cfg.neg_slope, cfg.ln_eps).reshape(-1)[0])

    # ---- phase C sharding: contiguous target ranges, row-balanced
    tot1 = int(start1[-1])
    tgb = [0]
    for c in range(1, NCORES):
        tgb.append(int(np.searchsorted(start1, tot1 * c // NCORES)))
    tgb.append(cfg.len_edges)

    bcols = seg_pc // P
    dcols = cfg.dcols
    sentinel_slot = P * cfg.t2 - 1
    U = n_cyc                      # targets >= U have no Edge2cycle rows
    assert int(cnt1[U:].sum()) == 0, "tail targets must be empty"
    tail_n = cfg.len_edges - U
    assert tail_n % NCORES == 0
    cst[5] = np.float32(tail_n // NCORES) * np.float32(g0) ** 2
    wpack[:, 4:20] = np.broadcast_to(cst, (P, 16))

    in_maps = []
    nt_list = []
    for c in range(NCORES):
        m = {}
        m["xT"] = np.ascontiguousarray(
            xT[:, c * cfg.toka:(c + 1) * cfg.toka]).astype(
                ml_dtypes.float8_e4m3 if cfg.a_fp8 else np.float16)
        m["wpack"] = wpack
        m["wblk8"] = wblk8
        m["gtab"] = g_tab

        # --- phase B stream
        lo, hi = c * seg_pc, (c + 1) * seg_pc
        glo, ghi = int(start0[lo]), int(start0[hi])
        e_sel = order0[glo:ghi]
        segl = (seg0s[glo:ghi] - lo)
        scounts = cnt0[lo:hi]
        sstarts = (start0[lo:hi] - glo)
        pos1 = tab_pos(seg1[e_sel]).astype(np.int64)
        st = _layout_stream(
            {"bp": (pce[e_sel], np.float32(-300.0)),
             "bpos": (pos1, np.int64(n_cyc))},
            segl, scounts, sstarts, cfg.t1)
        m["bp"] = st["bp"].astype(np.float16)
        m["bmsk"] = st["__mask"].astype(np.float16)
        m["bpos"] = st["bpos"].astype(np.int32)
        m["bidx"] = st["__end"].reshape(P, bcols).astype(np.int32)

        # --- phase C stream (own target range)
        lo1, hi1 = tgb[c], tgb[c + 1]
        g1lo, g1hi = int(start1[lo1]), int(start1[hi1])
        r_sel = order1[g1lo:g1hi]
        tgtl = (tgts[g1lo:g1hi] - lo1)
        tcounts = cnt1[lo1:hi1]
        tstarts = (start1[lo1:hi1] - g1lo)
        st2 = _layout_stream(
            {"cpos": (src[r_sel].astype(np.int64), np.int64(n_cyc))},
            tgtl, tcounts, tstarts, cfg.t2)
        # max-scan reset stream: 0 within a segment, -60000 at starts
        m["crst"] = ((st2["__mask"] - 1.0) * 60000.0).astype(np.float16)
        m["cpos"] = st2["cpos"].astype(np.int32)

        # --- phase D dense per-target streams (clipped at U) + one
        # synthetic om=0/wk=1 token at index n_t for the tail value
        hi_d = min(hi1, U)
        n_t = hi_d - lo1
        assert n_t + 1 <= P * dcols, (n_t, P * dcols)
        nt_list.append(n_t)
        om_pos = np.full(P * dcols, sentinel_slot, np.int64)
        om_pos[:n_t] = np.where(tcounts[:n_t] > 0, st2["__end"][:n_t],
                                sentinel_slot)
        m["ompos"] = om_pos.reshape(P, dcols).astype(np.int32)
        wkD = np.zeros(P * dcols, np.float32)
        wkD[:n_t] = wk[lo1:hi_d]
        wkD[n_t] = 1.0
        m["wkd"] = wkD.reshape(P, dcols).astype(np.float16)
        omm = np.zeros(P * dcols, np.float32)
        omm[:n_t] = (tcounts[:n_t] > 0).astype(np.float32)
        m["omm"] = omm.reshape(P, dcols).astype(np.float16)
        mv = np.zeros(P * dcols, np.float32)
        mv[:n_t] = 1.0
        m["mv"] = mv.reshape(P, dcols).astype(np.float16)
        in_maps.append(m)

    asm = {"tgb": tgb, "n_t": nt_list, "U": U}
    return in_maps, asm


def assemble_output(results, asm, cfg):
    U = asm["U"]
    out = np.empty(cfg.len_edges, np.float32)
    for c in range(NCORES):
        y = np.asarray(results[c]["y"]).reshape(-1)
        lo1 = asm["tgb"][c]
        hi_d = min(asm["tgb"][c + 1], U)
        out[lo1:hi_d] = y[:asm["n_t"][c]]
    yl = np.asarray(results[NCORES - 1]["y"]).reshape(-1)
    out[U:] = yl[asm["n_t"][NCORES - 1]]
    return out


# ---------------------------------------------------------------------------
# device program
# ---------------------------------------------------------------------------

def build_nc(cfg):
    import concourse.bass as bass
    import concourse.bacc as bacc
    import concourse.mybir as mybir
    import concourse.tile as tile
    from contextlib import ExitStack

    dt = mybir.dt
    f32, i32, f16 = dt.float32, dt.int32, dt.float16
    f8 = dt.float8e4
    hdt = f16 if cfg.mlp_bf16 else f32
    Alu = mybir.AluOpType
    Act = mybir.ActivationFunctionType

    n_cyc, od = cfg.n_cyc, cfg.out_dim
    seg_pc = cfg.seg_pc
    toka = cfg.toka
    t1, t2, mcols = cfg.t1, cfg.t2, cfg.mcols
    kch = od // P
    spr = toka // P
    bcols = seg_pc // P
    dcols = cfg.dcols
    nch_b = t1 // mcols
    nch_c = t2 // mcols
    groups = [list(range(NCORES))]

    nc = bacc.Bacc(trn_type="TRN2", num_devices=NCORES)

    def din(name, shape, dtype=f32):
        return nc.declare_dram_parameter(name, list(shape), dtype, False).ap()

    adt = f8 if cfg.a_fp8 else f16
    wcols = 4 + 16
    xT = din("xT", [od, toka], adt)
    wpack = din("wpack", [P, wcols])
    wblk8 = din("wblk8", [P, 4 * P], adt)
    gtab = din("gtab", [cfg.n_tab])
    bp = din("bp", [P, t1], f16)
    bmsk = din("bmsk", [P, t1], f16)
    bpos = din("bpos", [P, t1], i32)
    bidx = din("bidx", [P, bcols], i32)
    crst = din("crst", [P, t2], f16)
    cpos = din("cpos", [P, t2], i32)
    ompos = din("ompos", [P, dcols], i32)
    wkd = din("wkd", [P, dcols], f16)
    omm = din("omm", [P, dcols], f16)
    mv = din("mv", [P, dcols], f16)
    y_out = nc.declare_dram_parameter("y", [P * dcols], f32, True).ap()

    out_part = nc.dram_tensor("out_part", [toka], f16).ap()
    astat = nc.dram_tensor("astat", [toka * 4], f32).ap()
    out_tab = nc.dram_tensor("out_tab", [n_cyc + ELEM], f16,
                             addr_space="Shared").ap()
    bredW = nc.dram_tensor("bredW", [P * t1], f16).ap()
    bredU = nc.dram_tensor("bredU", [P * t1], f16).ap()
    out2_part = nc.dram_tensor("out2_part", [seg_pc], f16).ap()
    out2_tab = nc.dram_tensor("out2_tab", [n_cyc + ELEM], f16,
                              addr_space="Shared").ap()
    credM = nc.dram_tensor("credM", [P * t2], f32).ap()
    nsq_part = nc.dram_tensor("nsq_part", [16], f32).ap()
    nsq_tab = nc.dram_tensor("nsq_tab", [16], f32, addr_space="Shared").ap()

    def r2(ap_, p=P):
        return ap_.rearrange("(p c) -> p c", p=p)

    def col(ap_):
        return ap_.rearrange("(a b) -> a b", b=1)

    def _finish(ctx):
        ctx.close()
        return nc

    with ExitStack() as ctx:
        tc = ctx.enter_context(tile.TileContext(nc))
        cpool = ctx.enter_context(tc.tile_pool(name="cpool", bufs=1))
        sb = ctx.enter_context(tc.tile_pool(name="sb", bufs=2))
        sb3 = ctx.enter_context(tc.tile_pool(name="sb3", bufs=3))
        ps = ctx.enter_context(tc.tile_pool(name="ps", bufs=3, space="PSUM"))
        ps1 = ctx.enter_context(tc.tile_pool(name="ps1", bufs=1,
                                             space="PSUM"))

        def stt(out, in0, scalar, in1, op0, op1, accum=None):
            nc.vector.scalar_tensor_tensor(out=out, in0=in0, scalar=scalar,
                                           in1=in1, op0=op0, op1=op1,
                                           accum_out=accum)

        from concourse.tile import add_dep_helper as _adh
        loose = []

        def DMA(*a, **kw):
            inst = nc.sync.dma_start(*a, **kw)
            loose.append(inst)
            return inst

        def IDMA(*a, **kw):
            inst = nc.gpsimd.indirect_dma_start(*a, **kw)
            loose.append(inst)
            return inst

        def CC(*a, **kw):
            inst = nc.gpsimd.collective_compute(*a, **kw)
            loose.append(inst)
            return inst

        def fence():
            items = list(loose)
            loose.clear()
            if not items:
                return
            for eng in (nc.vector, nc.scalar, nc.tensor, nc.gpsimd,
                        nc.sync):
                for j in range(0, len(items), 2):
                    nop = eng.nop()
                    for d in items[j:j + 2]:
                        _adh(nop.ins, d.ins, sync=True, reason="fence")
            tc.no_sync_barrier()

        # ---- constants: one DMA
        wp = cpool.tile([P, wcols], f32, tag="wp")
        DMA(out=wp[:], in_=wpack[:, :])
        wblk_sb = cpool.tile([P, 4 * P], adt, tag="wblk")
        DMA(out=wblk_sb[:], in_=wblk8[:, :])
        ow2_sb = cpool.tile([P, 4], f16, tag="ow2")
        nc.vector.tensor_copy(out=ow2_sb[:], in_=wp[:, 0:4])
        cst = wp[:, 4:20]
        sent0 = cpool.tile([1, ELEM], f16, tag="sent0")
        nc.gpsimd.memset(sent0[:], 0.0)
        sent1 = cpool.tile([1, ELEM], f16, tag="sent1")
        nc.gpsimd.memset(sent1[:], -300.0)

        # chunked stream gather: [P, w] of table[pos]
        def gather_chunk(pool, tab_col, pos_dram_sl, w, tag, vdt=f16):
            pos_sb = pool.tile([P, w], i32, tag=tag + "_pos")
            DMA(out=pos_sb[:], in_=pos_dram_sl)
            val = pool.tile([P, w], vdt, tag=tag + "_val")
            IDMA(out=val[:], out_offset=None, in_=tab_col,
                 in_offset=bass.IndirectOffsetOnAxis(ap=pos_sb[:], axis=0))
            return val

        fence()

        def pipeline():
            # ============================================================
            # Phase A (transposed: features on partitions, tokens free)
            # ============================================================
            TA = 512
            nchA = toka // TA
            xTk = xT.rearrange("(k p) c -> p k c", k=kch)
            astat2 = astat.rearrange("(k r c) -> k r c", r=2, c=2 * TA)
            with tc.tile_pool(name="apool", bufs=3) as apl, \
                    tc.tile_pool(name="apsum", bufs=2, space="PSUM") as aps, \
                    tc.tile_pool(name="apsq", bufs=2, space="PSUM") as apq:
                wblkv = wblk_sb[:].rearrange("p (two m) -> p two m", two=2)
                for k in range(nchA):
                    xt = apl.tile([P, 2 * TA], adt, tag="xt")
                    DMA(out=xt[:], in_=xTk[:, :, k * TA:(k + 1) * TA])
                    xtv = xt[:].rearrange("p (two t) -> p two t", two=2)
                    stq = apq.tile([2, 2 * TA], f32, tag="stq")
                    for jh in range(2):
                        zp = aps.tile([P, TA], f32, tag="zp")
                        if cfg.a_fp8:
                            nc.tensor.matmul(
                                out=zp[:],
                                lhsT=wblkv[:, :, jh * P:(jh + 1) * P],
                                rhs=xtv,
                                perf_mode=mybir.MatmulPerfMode.DoubleRow,
                                start=True, stop=True)
                        else:
                            for kin in range(2):
                                b = (kin * 2 + jh) * P
                                nc.tensor.matmul(
                                    out=zp[:],
                                    lhsT=wblk_sb[:, b:b + P],
                                    rhs=xt[:, kin * TA:(kin + 1) * TA],
                                    start=(kin == 0), stop=(kin == 1))
                        hh = apl.tile([P, 2 * TA], f16, tag="hh")
                        nc.scalar.activation(
                            out=hh[:, 0:TA], in_=zp[:], func=Act.Lrelu,
                            alpha=cfg.neg_slope,
                            scale=(1.0 / 32.0) if cfg.a_fp8 else 1.0)
                        stt(hh[:, TA:2 * TA], hh[:, 0:TA], 1.0,
                            hh[:, 0:TA], Alu.mult, Alu.mult)
                        for hb in range(2):
                            nc.tensor.matmul(
                                out=stq[:, hb * TA:(hb + 1) * TA],
                                lhsT=ow2_sb[:, 2 * jh:2 * jh + 2],
                                rhs=hh[:, hb * TA:(hb + 1) * TA],
                                start=(jh == 0), stop=(jh == 1))
                    stg = apl.tile([2, 2 * TA], f32, tag="stg")
                    if k % 2 == 0:
                        nc.scalar.activation(out=stg[:], in_=stq[:],
                                             func=Act.Copy)
                    else:
                        nc.vector.tensor_copy(out=stg[:], in_=stq[:])
                    DMA(out=astat2[k, :, :], in_=stg[:])
            fence()
            # reload stats dense: [nchA, TA] grids (token = k*TA + t)
            HN = nchA
            sq_d = cpool.tile([HN, 2 * TA], f32, tag="sq_d")
            DMA(out=sq_d[:], in_=astat2[:, 0, :])
            d_d = cpool.tile([HN, TA], f32, tag="d_d")
            DMA(out=d_d[:], in_=astat2[:, 1, 0:TA])
            # tail: per-token LN fold on dense [nchA, TA] grids
            Sv, Qv, Dv = sq_d[:, 0:TA], sq_d[:, TA:2 * TA], d_d[:, 0:TA]
            mu = cpool.tile([HN, TA], f32, tag="a_mu")
            nc.vector.tensor_scalar_mul(out=mu[:], in0=Sv, scalar1=1.0 / od)
            sqm = cpool.tile([HN, TA], f32, tag="a_sqm")
            stt(sqm[:], mu[:], 1.0, mu[:], Alu.mult, Alu.mult)
            var = cpool.tile([HN, TA], f32, tag="a_var")
            stt(var[:], Qv, 1.0 / od, sqm[:], Alu.mult, Alu.subtract)
            sd = cpool.tile([HN, TA], f32, tag="a_sd")
            nc.scalar.activation(out=sd[:], in_=var[:], func=Act.Sqrt,
                                 bias=cst[0:HN, 4:5])
            rsd = cpool.tile([HN, TA], f32, tag="a_rsd")
            nc.vector.reciprocal(out=rsd[:], in_=sd[:])
            num = cpool.tile([HN, TA], f32, tag="a_num")
            stt(num[:], mu[:], cst[0:HN, 0:1], Dv, Alu.mult, Alu.add)
            out_sb = cpool.tile([HN, TA], f16, tag="out_sb")
            stt(out_sb[:], num[:], 1.0, rsd[:], Alu.mult, Alu.mult)
            nc.vector.tensor_scalar(out=out_sb[:], in0=out_sb[:],
                                    scalar1=cst[0:HN, 1:2], scalar2=None,
                                    op0=Alu.add)
            DMA(out=r2(out_part, p=HN), in_=out_sb[:])

            # ---- prefetch B/C/D streams; exp(leaky(bp)) overlaps phase A
            bpos_t = []
            for i in range(nch_b):
                pt = cpool.tile([P, mcols], i32, tag=f"bpos_t{i}")
                DMA(out=pt[:], in_=bpos[:, i * mcols:(i + 1) * mcols])
                bpos_t.append(pt)
            bpc_full = cpool.tile([P, t1], f16, tag="bpc_full")
            DMA(out=bpc_full[:], in_=bp[:, :])
            bmsk_sb = cpool.tile([P, t1], f16, tag="bmsk_sb")
            DMA(out=bmsk_sb[:], in_=bmsk[:, :])
            cpos_t = []
            for i in range(nch_c):
                pt = cpool.tile([P, mcols], i32, tag=f"cpos_t{i}")
                DMA(out=pt[:], in_=cpos[:, i * mcols:(i + 1) * mcols])
                cpos_t.append(pt)
            crst_sb = cpool.tile([P, t2], f16, tag="crst_sb")
            DMA(out=crst_sb[:], in_=crst[:, :])
            bidx_sb = cpool.tile([P, bcols], i32, tag="bidx_sb")
            DMA(out=bidx_sb[:], in_=bidx[:, :])
            ompos_sb = cpool.tile([P, dcols], i32, tag="ompos_sb")
            DMA(out=ompos_sb[:], in_=ompos[:, :])
            wkd_sb = cpool.tile([P, dcols], f16, tag="wkd_sb")
            DMA(out=wkd_sb[:], in_=wkd[:, :])
            omm_sb = cpool.tile([P, dcols], f16, tag="omm_sb")
            DMA(out=omm_sb[:], in_=omm[:, :])
            mv_sb = cpool.tile([P, dcols], f16, tag="mv_sb")
            DMA(out=mv_sb[:], in_=mv[:, :])
            stt(bpc_full[:], bpc_full[:], cfg.neg_slope, bpc_full[:],
                Alu.mult, Alu.max)
            nc.scalar.activation(out=bpc_full[:], in_=bpc_full[:],
                                 func=Act.Exp)

            CC("AllGather", Alu.bypass, replica_groups=groups,
               ins=[out_part[:]], outs=[out_tab[0:n_cyc]])
            DMA(out=r2(out_tab[n_cyc:n_cyc + ELEM], p=1), in_=sent0[:])
            fence()

            if "b" not in cfg.phases:
                return False

            # ============================================================
            # Phase B
            # ============================================================
            with tc.tile_pool(name="bpool", bufs=3) as bpl, \
                    tc.tile_pool(name="bstage", bufs=1) as bst:
                wstage = bst.tile([P, t1], f16, tag="wstage")
                ustage = bst.tile([P, t1], f16, tag="ustage")
                for i in range(nch_b):
                    sl = slice(i * mcols, (i + 1) * mcols)
                    val = bpl.tile([P, mcols], f16, tag="bg_val")
                    IDMA(out=val[:], out_offset=None, in_=col(out_tab[:]),
                         in_offset=bass.IndirectOffsetOnAxis(
                             ap=bpos_t[i][:], axis=0))
                    uval = bpl.tile([P, mcols], f16, tag="uval")
                    nc.vector.tensor_tensor(out=uval[:],
                                            in0=bpc_full[:, sl],
                                            in1=val[:], op=Alu.mult)
                    nc.vector.tensor_tensor_scan(
                        out=wstage[:, sl], data0=bmsk_sb[:, sl],
                        data1=bpc_full[:, sl],
                        initial=(0.0 if i == 0 else
                                 wstage[:, i * mcols - 1:i * mcols]),
                        op0=Alu.mult, op1=Alu.add)
                    nc.vector.tensor_tensor_scan(
                        out=ustage[:, sl], data0=bmsk_sb[:, sl],
                        data1=uval[:],
                        initial=(0.0 if i == 0 else
                                 ustage[:, i * mcols - 1:i * mcols]),
                        op0=Alu.mult, op1=Alu.add)
                DMA(out=r2(bredW), in_=wstage[:])
                DMA(out=r2(bredU), in_=ustage[:])
                fence()

            # ---- segment extraction: dW/dU at end slots, then o2
            dW = cpool.tile([P, bcols], f16, tag="dW")
            IDMA(out=dW[:], out_offset=None, in_=col(bredW[:]),
                 in_offset=bass.IndirectOffsetOnAxis(ap=bidx_sb[:], axis=0))
            dU = cpool.tile([P, bcols], f16, tag="dU")
            IDMA(out=dU[:], out_offset=None, in_=col(bredU[:]),
                 in_offset=bass.IndirectOffsetOnAxis(ap=bidx_sb[:], axis=0))
            den = cpool.tile([P, bcols], f32, tag="den")
            nc.vector.tensor_scalar_add(out=den[:], in0=dW[:],
                                        scalar1=1e-12)
            nc.vector.reciprocal(out=den[:], in_=den[:])
            o2 = cpool.tile([P, bcols], f16, tag="o2")
            nc.vector.tensor_tensor(out=o2[:], in0=den[:], in1=dU[:],
                                    op=Alu.mult)
            DMA(out=r2(out2_part), in_=o2[:])
            fence()
            CC("AllGather", Alu.bypass, replica_groups=groups,
               ins=[out2_part[:]], outs=[out2_tab[0:n_cyc]])
            DMA(out=r2(out2_tab[n_cyc:n_cyc + ELEM], p=1), in_=sent1[:])
            fence()

            if "c" not in cfg.phases:
                return False

            # ============================================================
            # Phase C
            # ============================================================
            with tc.tile_pool(name="cpool2", bufs=3) as cpl, \
                    tc.tile_pool(name="cstage", bufs=1) as cstg:
                mstage = cstg.tile([P, t2], f32, tag="mstage")
                for i in range(nch_c):
                    sl = slice(i * mcols, (i + 1) * mcols)
                    val = cpl.tile([P, mcols], f16, tag="cg_val")
                    IDMA(out=val[:], out_offset=None, in_=col(out2_tab[:]),
                         in_offset=bass.IndirectOffsetOnAxis(
                             ap=cpos_t[i][:], axis=0))
                    nc.vector.tensor_tensor_scan(
                        out=mstage[:, sl], data0=crst_sb[:, sl],
                        data1=val[:],
                        initial=(0.0 if i == 0 else
                                 mstage[:, i * mcols - 1:i * mcols]),
                        op0=Alu.add, op1=Alu.max)
                DMA(out=r2(credM), in_=mstage[:])
                fence()

            if "d" not in cfg.phases:
                return False

            # ============================================================
            # Phase D: angle-table MLP2 over the dense target range
            # ============================================================
            omr = cpool.tile([P, dcols], f32, tag="omr")
            IDMA(out=omr[:], out_offset=None, in_=col(credM[:]),
                 in_offset=bass.IndirectOffsetOnAxis(ap=ompos_sb[:], axis=0))
            fence()

            om = cpool.tile([P, dcols], f32, tag="om")
            nc.vector.tensor_tensor(out=om[:], in0=omr[:], in1=omm_sb[:],
                                    op=Alu.mult)
            aom = cpool.tile([P, dcols], f32, tag="aom")
            stt(aom[:], om[:], -1.0, om[:], Alu.mult, Alu.max)
            s_ = cpool.tile([P, dcols], f32, tag="s_")
            stt(s_[:], aom[:], 1e-12, wkd_sb[:], Alu.add, Alu.add)
            rs_ = cpool.tile([P, dcols], f32, tag="rs_")
            nc.vector.reciprocal(out=rs_[:], in_=s_[:])
            u2 = cpool.tile([P, dcols], f32, tag="u2")
            stt(u2[:], wkd_sb[:], 0.5, rs_[:], Alu.mult, Alu.mult)
            mge = cpool.tile([P, dcols], f32, tag="mge")
            nc.vector.tensor_scalar(out=mge[:], in0=om[:], scalar1=0.0,
                                    scalar2=None, op0=Alu.is_ge)
            wng = cpool.tile([P, dcols], f32, tag="wng")
            nc.vector.tensor_scalar(out=wng[:], in0=u2[:], scalar1=-2.0,
                                    scalar2=1.0, op0=Alu.mult, op1=Alu.add)
            tq = cpool.tile([P, dcols], f32, tag="tq")
            nc.vector.tensor_tensor(out=tq[:], in0=mge[:], in1=wng[:],
                                    op=Alu.mult)
            tv = cpool.tile([P, dcols], f32, tag="tv")
            stt(tv[:], u2[:], 1.0, wng[:], Alu.mult, Alu.add)
            nc.vector.tensor_tensor(out=tv[:], in0=tv[:], in1=tq[:],
                                    op=Alu.subtract)
            nc.vector.tensor_scalar_mul(out=tv[:], in0=tv[:],
                                        scalar1=float(cfg.n_tab - 1))
            tix = cpool.tile([P, dcols], i32, tag="tix")
            nc.vector.tensor_copy(out=tix[:], in_=tv[:])
            y0 = cpool.tile([P, dcols], f32, tag="y0")
            IDMA(out=y0[:], out_offset=None, in_=col(gtab[:]),
                 in_offset=bass.IndirectOffsetOnAxis(ap=tix[:], axis=0))
            fence()
            ym = cpool.tile([P, dcols], f32, tag="ym")
            nc.vector.tensor_tensor(out=ym[:], in0=y0[:], in1=mv_sb[:],
                                    op=Alu.mult)

            # ---- global L2 norm
            ssq = cpool.tile([P, 1], f32, tag="ssq")
            scr = cpool.tile([P, dcols], f32, tag="scr")
            stt(scr[:], ym[:], 1.0, ym[:], Alu.mult, Alu.mult,
                accum=ssq[:, 0:1])
            ones = cpool.tile([P, 1], f32, tag="ones")
            nc.gpsimd.memset(ones[:], 1.0)
            sred = ps1.tile([1, 1], f32, tag="sred")
            nc.tensor.matmul(out=sred[:], lhsT=ones[:], rhs=ssq[:],
                             start=True, stop=True)
            nsq_sb = cpool.tile([1, 16], f32, tag="nsq_sb")
            nc.gpsimd.memset(nsq_sb[:], 0.0)
            nc.vector.tensor_scalar(out=nsq_sb[:, 0:1], in0=sred[:],
                                    scalar1=cst[0:1, 5:6], scalar2=None,
                                    op0=Alu.add)
            DMA(out=r2(nsq_part, p=1), in_=nsq_sb[:])
            fence()
            CC("AllReduce", Alu.add, replica_groups=groups,
               ins=[nsq_part[:]], outs=[nsq_tab[:]])
            fence()
            return ym

        def stub_outputs():
            dumf = cpool.tile([P, dcols], f32, tag="dumf")
            nc.gpsimd.memset(dumf[:], 0.0)
            DMA(out=r2(y_out), in_=dumf[:])

        res = None
        for _rep in range(cfg.repeat):
            res = pipeline()
        if res is False:
            stub_outputs()
            return _finish(ctx)
        ym = res

        nrm = cpool.tile([1, 1], f32, tag="nrm")
        DMA(out=nrm[:], in_=r2(nsq_tab[0:1], p=1))
        nc.scalar.activation(out=nrm[:], in_=nrm[:], func=Act.Sqrt)
        nc.vector.tensor_scalar_max(out=nrm[:], in0=nrm[:], scalar1=1e-12)
        nc.vector.reciprocal(out=nrm[:], in_=nrm[:])
        ones_row = cpool.tile([1, P], f32, tag="ones_row")
        nc.gpsimd.memset(ones_row[:], 1.0)
        rn_ps = ps1.tile([P, 1], f32, tag="rn_ps")
        nc.tensor.matmul(out=rn_ps[:], lhsT=ones_row[:], rhs=nrm[:],
                         start=True, stop=True)
        nrn_sb = cpool.tile([P, 1], f32, tag="nrn_sb")
        nc.vector.tensor_scalar(out=nrn_sb[:], in0=rn_ps[:], scalar1=-1.0,
                                scalar2=None, op0=Alu.mult)
        # sigmoid(x) = 1/(1+exp(-x)) via Exp + HW reciprocal
        nc.scalar.activation(out=ym[:], in_=ym[:], func=Act.Exp,
                             scale=nrn_sb[:, 0:1])
        nc.vector.tensor_scalar_add(out=ym[:], in0=ym[:], scalar1=1.0)
        nc.vector.reciprocal(out=ym[:], in_=ym[:])
        DMA(out=r2(y_out), in_=ym[:])

    return nc


# ---------------------------------------------------------------------------
# entry point
# ---------------------------------------------------------------------------

_NC_CACHE = {}


def _get_nc(cfg):
    key = (cfg.n_cyc, cfg.e_cc, cfg.len_edges, cfg.t1, cfg.t2,
           cfg.dcols, cfg.phases, cfg.repeat, cfg.a_fp8)
    if key not in _NC_CACHE:
        nc = build_nc(cfg)
        if not nc.is_finalized():
            nc.finalize()
        _NC_CACHE[key] = nc
    return _NC_CACHE[key]


def run(inputs, cfg=None, trace=False):
    from concourse.bass_utils import run_bass_kernel_spmd
    cfg = cfg or Cfg()
    in_maps, asm = host_prepare(inputs, cfg)
    nc = _get_nc(cfg)
    res = run_bass_kernel_spmd(nc, in_maps, core_ids=list(range(NCORES)),
                               trace=trace)
    return assemble_output(res.results, asm, cfg), res


def kernel(**inputs):
    out, _ = run(inputs)
    return out
